# revision 37
# baseline (speedup 1.0000x reference)
"""Trainium2 Bass kernel for ContextVectorNN (Bahdanau-style attention scorer).

Reference computation (per batch b):
  ts = text[b].T                              # [FT, L]
  x  = concat([ts, ss[b] bcast over L, cov])  # [C=1025, L]
  h  = tanh(W1 @ x + b1)                      # [H, L]
  logits = W2 @ h + b2                        # [1, L]
  att = softmax(mask(logits, len_b))          # [1, L]
  ctx = ts @ att.T                            # [FT]

Key structure used on device:
  - The summary channels are constant over L, so W1s @ ss[b] + b1 collapses to a
    per-batch bias vector (tiny; prepared host-side with the weights).
  - hT[l, h] = sum_c text[l, c] * W1T[c, h] via bf16 PE matmuls: transposed
    text tiles are the stationary operand, W1T chunks the moving operand.
  - coverage*w1c + bias are rank-1 in [L, H]: folded into the same PSUM
    accumulation as one extra K=2 matmul per L-tile, with the [cov; ones] rows
    and [w1c; bias] columns shipped inside the transposed-text input.
  - logits = sum_h W2[h]*tanh(hT[l, h]): tanh on ScalarE (PSUM -> SBUF bf16),
    then an in-place multiply by broadcast W2 and a free-axis reduce on DVE.
  - softmax uses a compile-time upper bound Bc >= max logit (exp never
    overflows), so no cross-partition max pass is needed; masked lanes are
    multiplied by 0 (exact zeros, matching softmax(-inf)). The denominator's
    cross-partition sum and broadcast are tiny PE matmuls with a ones vector.
  - attention beyond each batch's length is exactly 0, so each core only
    processes ceil(group_max_len/128)*128 positions per batch; batches are
    sorted by length and dealt one-per-core so the 8 cores stay balanced.

Sharding: data-parallel over batch across 8 cores (4 batches per core, one per
"slot"); weights replicated.

Implementation note: this walrus build allows very few semaphore waits per
instruction (1 for DMA/DVE ops, 2 for ScalarE), so the program is structured so
every instruction needs at most that: constants live in two packed arrays
observed once per engine at startup, hot buffers are manually multi-buffered
(no tile-pool slot releases), and cheap "observer" ops absorb cross-engine
dependencies before buffer reuse.
"""

import sys

sys.path.insert(0, "/opt/trn_rl_repo")

import numpy as np
import ml_dtypes

import concourse.bass as bass
import concourse.mybir as mybir
import concourse.tile as tile
from concourse.bass_utils import run_bass_kernel_spmd
from concourse.vector_clock import ScopedClock


def _spread_drain_and_barrier(self, tick_clock, wait_clock):
    """Replacement for TileContext._drain_and_barrier: this walrus build
    rejects instructions with more than one sync wait, and the kernel-tail
    drain normally carries the whole global clock. Spread those waits over
    individual one-wait NOPs on the sync engine first."""
    nc = self.nc
    probe = nc.sync.nop()
    wait_clock.add_sem_waits(probe.ins, ScopedClock({None: tick_clock.global_clock}))
    si = probe.ins.sync_info
    waits = list(si.on_wait or []) if si is not None else []
    if len(waits) > 1:
        probe.ins.sync_info = mybir.SyncInfo(
            on_wait=[waits[0]], on_update=list(si.on_update or [])
        )
        for w in waits[1:]:
            ex = nc.sync.nop()
            ex.ins.sync_info = mybir.SyncInfo(on_wait=[w], on_update=[])
    # SP executed the probe/extra NOPs in order, so the drain itself needs
    # no waits of its own.
    nc.sync.drain()
    nc.all_engine_barrier()
    assert self.sems is not None
    popped = nc._tile_sem_poison_stack.pop()
    assert popped is self._sem_poison
    nc.clear_and_free_semaphores(list(self.sems.allocated().values()))
    nc.all_engine_barrier()


tile.TileContext._drain_and_barrier = _spread_drain_and_barrier

BF16 = mybir.dt.bfloat16
F32 = mybir.dt.float32

N_CORES = 8
FT = 512
H = 512
KC = 4  # number of 128-channel chunks of the text features

# bf16 consts pack layout (columns): [w1tT chunks 0..3 | w2bc]
CB_W1T = 0
CB_W2 = KC * H
CB_COLS = KC * H + H
# f32 consts pack layout: [ones block 129 | ident 128 | mask 4*32 | bexp col]
CF_ONES = 0  # cols [0, 129) all ones: col 0 = ones col, row 0 = ones row
CF_IDENT = 129
CF_MASK = 129 + 128
CF_BEXP = 129 + 128 + 4 * 32
CF_COLS = 129 + 128 + 4 * 32 + 1


def build_program(Lps):
    """Build the SPMD Bass program. Lps: per-slot padded lengths (mult of 128)."""
    nslots = len(Lps)
    nc = bass.Bass()

    text_d = [
        nc.dram_tensor(f"text{j}", [Lps[j], FT], BF16, kind="ExternalInput")
        for j in range(nslots)
    ]
    # ttex: transposed text, rows = channels
    ttex_d = [
        nc.dram_tensor(f"ttex{j}", [512, Lps[j]], BF16, kind="ExternalInput")
        for j in range(nslots)
    ]
    # cov2: rows 32j   = [coverage_j | w1c], rows 32j+1 = [ones | bias_j]
    cov2_d = nc.dram_tensor("cov2", [128, 4608], BF16, kind="ExternalInput")
    cb_d = nc.dram_tensor("constsb", [128, CB_COLS], BF16, kind="ExternalInput")
    cf_d = nc.dram_tensor("constsf", [128, CF_COLS], F32, kind="ExternalInput")
    ctx_d = nc.dram_tensor("ctx", [nslots, FT], BF16, kind="ExternalOutput")
    att_d = [
        nc.dram_tensor(f"att{j}", [Lps[j] // 128, 128], F32, kind="ExternalOutput")
        for j in range(nslots)
    ]

    with tile.TileContext(nc) as tc:
        with (
            tc.tile_pool(name="consts", bufs=1) as consts,
            tc.tile_pool(name="data", bufs=1) as datap,
            tc.tile_pool(name="pfix", bufs=1, space="PSUM") as pfix,
        ):
            cb = consts.tile([128, CB_COLS], BF16)
            nc.sync.dma_start(out=cb[:], in_=cb_d[:])
            cf = consts.tile([128, CF_COLS], F32)
            nc.sync.dma_start(out=cf[:], in_=cf_d[:])
            cov2 = consts.tile([128, 4608], BF16)
            nc.sync.dma_start(out=cov2[:], in_=cov2_d[:])

            def w1t_sb(k):
                return cb[:, CB_W1T + k * H : CB_W1T + (k + 1) * H]

            w2bc_sb = cb[:, CB_W2 : CB_W2 + H]
            ones_col = cf[:, CF_ONES : CF_ONES + 1]
            ones_row = cf[0:1, CF_ONES + 1 : CF_ONES + 129]
            ident_sb = cf[:, CF_IDENT : CF_IDENT + 128]
            bexp_sb = cf[:, CF_BEXP : CF_BEXP + 1]

            def mask_sb(j, nt):
                return cf[:, CF_MASK + j * 32 : CF_MASK + j * 32 + nt]

            # fixed (manually rotated) buffers — avoids tile-pool release
            # semaphores, which would exceed per-instruction wait limits
            NH = 4
            htan = [consts.tile([128, H], BF16, name=f"ht{i}", tag=f"ht{i}") for i in range(NH)]
            NP = 3
            hps = [pfix.tile([128, H], F32, name=f"hp{i}", tag=f"hp{i}") for i in range(NP)]
            ctxps = [pfix.tile([1, FT], F32, name=f"cx{i}", tag=f"cx{i}") for i in range(2)]
            # shared small psum banks:
            # psA: [atT 0:128 | denom 128:160 | observers 162:166]
            psA = pfix.tile([128, 512], F32, tag="psA")
            psB = pfix.tile([128, 512], F32, tag="psB")  # denominator broadcast

            # engine observers: each engine waits once on the const DMAs so
            # later instructions elide those deps
            nc.tensor.matmul(
                psA[0:1, 162:163], cb[0:1, 0:1], cb[0:1, 0:1], start=True, stop=True
            )
            nc.tensor.matmul(
                psA[0:1, 163:164], cf[0:1, 0:1], cf[0:1, 0:1], start=True, stop=True
            )
            nc.tensor.matmul(
                psA[0:1, 166:167], cov2[0:1, 0:1], cov2[0:1, 0:1],
                start=True, stop=True,
            )
            scr_dve = consts.tile([1, 8], F32)
            nc.vector.tensor_copy(scr_dve[0:1, 0:1], cb[0:1, 0:1])
            nc.vector.tensor_copy(scr_dve[0:1, 1:2], cf[0:1, 0:1])
            scr_act = consts.tile([1, 128], F32)
            nc.scalar.activation(
                scr_act[0:1, 0:1], cf[0:1, 0:1], mybir.ActivationFunctionType.Tanh
            )
            nc.scalar.activation(
                scr_act[0:1, 1:2], cf[0:1, 0:1], mybir.ActivationFunctionType.Exp
            )

            ti = 0  # global L-tile counter for htan/hps rotation
            red_hist = []  # (logits tile, col) per completed DVE reduce
            for j in range(nslots):
                nt = Lps[j] // 128
                Lp = Lps[j]

                # A) natural-layout text (context-matmul rhs); fresh tile
                nat = datap.tile(
                    [128, nt, FT], BF16, name=f"nat{j}", tag=f"nat{j}"
                )
                nc.sync.dma_start(
                    out=nat[:],
                    in_=text_d[j][:].rearrange("(t p) c -> p t c", p=128),
                )
                # B) transposed text chunks; fresh tile
                tsT = datap.tile(
                    [128, KC, Lp], BF16, name=f"tsT{j}", tag=f"tsT{j}"
                )
                nc.sync.dma_start(
                    out=tsT[:],
                    in_=ttex_d[j][:].rearrange("(k p) l -> p k l", p=128),
                )

                # PE observers for this slot's data DMAs
                nc.tensor.matmul(
                    psA[0:1, 164:165], nat[0:1, 0, 0:1], nat[0:1, 0, 0:1],
                    start=True, stop=True,
                )
                nc.tensor.matmul(
                    psA[0:1, 165:166], tsT[0:1, 0, 0:1], tsT[0:1, 0, 0:1],
                    start=True, stop=True,
                )

                logits = consts.tile([128, 32], F32, tag=f"lg{j}")

                for t in range(nt):
                    hp = hps[ti % NP]
                    ht = htan[ti % NH]
                    for k in range(KC):
                        nc.tensor.matmul(
                            hp[:],
                            tsT[:, k, t * 128 : (t + 1) * 128],
                            w1t_sb(k),
                            start=(k == 0),
                            stop=False,
                        )
                    nc.tensor.matmul(
                        hp[:],
                        cov2[32 * j : 32 * j + 2, t * 128 : (t + 1) * 128],
                        cov2[32 * j : 32 * j + 2, Lp : Lp + H],
                        start=False,
                        stop=True,
                        tile_position=(32 * j, 0),
                    )
                    if ti >= NH:
                        # make ACT observe the DVE tick that released ht:
                        # reading the reduce output of tile ti-2 (>= ti-NH)
                        plg, pt = red_hist[ti - 2]
                        col = 2 + (ti % 126)
                        nc.scalar.activation(
                            scr_act[0:1, col : col + 1],
                            plg[0:1, pt : pt + 1],
                            mybir.ActivationFunctionType.Copy,
                        )
                    nc.scalar.activation(
                        ht[:], hp[:], mybir.ActivationFunctionType.Tanh
                    )
                    # in-place: ht *= w2 (broadcast rows), then row-sum
                    nc.vector.tensor_tensor(
                        ht[:], ht[:], w2bc_sb, mybir.AluOpType.mult
                    )
                    nc.vector.tensor_reduce(
                        logits[:, t : t + 1],
                        ht[:],
                        axis=mybir.AxisListType.X,
                        op=mybir.AluOpType.add,
                    )
                    red_hist.append((logits, t))
                    ti += 1

                # ---- softmax tail (masked, bounded-exp)
                expv = consts.tile([128, 32], F32, tag=f"ex{j}")
                nc.scalar.activation(
                    expv[:, 0:nt],
                    logits[:, 0:nt],
                    mybir.ActivationFunctionType.Exp,
                    bias=bexp_sb,
                )
                attw = consts.tile([128, 32], F32, tag=f"aw{j}")
                nc.vector.tensor_tensor(
                    attw[:, 0:nt], expv[:, 0:nt], mask_sb(j, nt),
                    mybir.AluOpType.mult,
                )
                nc.tensor.matmul(
                    psA[0:1, 128 : 128 + nt], ones_col, attw[:, 0:nt],
                    start=True, stop=True,
                )
                den = consts.tile([1, 2], F32, tag=f"dn{j}")
                nc.vector.tensor_reduce(
                    den[0:1, 0:1], psA[0:1, 128 : 128 + nt],
                    axis=mybir.AxisListType.X, op=mybir.AluOpType.add,
                )
                nc.tensor.matmul(
                    psB[:, 0:1], ones_row, den[0:1, 0:1], start=True, stop=True
                )
                rcp = consts.tile([128, 2], F32, tag=f"rc{j}")
                nc.vector.reciprocal(rcp[:, 0:1], psB[:, 0:1])
                nc.vector.tensor_tensor(
                    attw[:, 0:nt],
                    attw[:, 0:nt],
                    rcp[:, 0:1].to_broadcast((128, nt)),
                    mybir.AluOpType.mult,
                )
                attb = consts.tile([128, 32], BF16, tag=f"ab{j}")
                nc.vector.tensor_copy(attb[:, 0:nt], attw[:, 0:nt])

                # attention out: PE-transpose [128, nt] -> [nt, 128], then DMA
                nc.tensor.matmul(
                    psA[0:nt, 0:128], attw[:, 0:nt], ident_sb,
                    is_transpose=True, start=True, stop=True,
                )
                atT = consts.tile([32, 128], F32, tag=f"at{j}")
                nc.vector.tensor_copy(atT[0:nt, :], psA[0:nt, 0:128])
                nc.gpsimd.dma_start(out=att_d[j][:], in_=atT[0:nt, :])

                # context: ctx[c] = sum_l att[l] * text[l, c]
                cx = ctxps[j % 2]
                for t in range(nt):
                    nc.tensor.matmul(
                        cx[:],
                        attb[:, t : t + 1],
                        nat[:, t, :],
                        start=(t == 0),
                        stop=(t == nt - 1),
                    )
                cxs = consts.tile([1, FT], BF16, tag=f"cs{j}")
                nc.vector.tensor_copy(cxs[:], cx[:])
                nc.gpsimd.dma_start(out=ctx_d[j : j + 1, :], in_=cxs[:])

    return nc


def prepare(text_states, summary_current_state, coverage, W1, b1, W2, b2, text_length):
    B, L, ft = text_states.shape
    assert (ft, L) == (FT, 4096) and B == 32

    text_states = np.asarray(text_states, dtype=np.float32)
    summary_current_state = np.asarray(summary_current_state, dtype=np.float32)
    coverage = np.asarray(coverage, dtype=np.float32)
    W1 = np.asarray(W1, dtype=np.float32)
    b1 = np.asarray(b1, dtype=np.float32)
    W2 = np.asarray(W2, dtype=np.float32)
    b2 = np.asarray(b2, dtype=np.float32)
    lens = np.asarray(text_length).astype(np.int64)

    nslots = B // N_CORES

    # length-sorted assignment: slot j holds ranks [8j, 8j+8), one per core
    order = np.argsort(-lens, kind="stable")
    assign = order.reshape(nslots, N_CORES)  # assign[j, i] = batch of core i slot j
    Lps = [
        max(128, int(np.ceil(lens[assign[j]].max() / 128.0) * 128))
        for j in range(nslots)
    ]

    # softmax upper bound: |logit| <= sum|W2| with margin; b2 folded into exp
    Bc = float(np.abs(W2).sum()) + 1.0
    bexp = float(b2.reshape(-1)[0]) - Bc

    nc = build_program(Lps)

    # per-batch bias vectors (tiny): bias_b = W1s @ ss_b + b1
    W1t = W1[:, :FT]
    W1s = W1[:, FT : FT + FT]
    w1c = W1[:, -1]
    biases = summary_current_state @ W1s.T + b1[None, :]  # [B, H] fp32
    w1c_bf = w1c.astype(ml_dtypes.bfloat16)

    constsf = np.zeros((128, CF_COLS), np.float32)
    constsf[:, CF_ONES : CF_ONES + 129] = 1.0
    constsf[:, CF_IDENT : CF_IDENT + 128] = np.eye(128, dtype=np.float32)
    constsf[:, CF_BEXP] = bexp

    constsb = np.zeros((128, CB_COLS), ml_dtypes.bfloat16)
    constsb[:, CB_W1T : CB_W1T + KC * H] = (
        np.ascontiguousarray(W1t.T)
        .reshape(KC, 128, H)
        .transpose(1, 0, 2)
        .reshape(128, KC * H)
        .astype(ml_dtypes.bfloat16)
    )
    constsb[:, CB_W2 : CB_W2 + H] = np.broadcast_to(W2.reshape(1, H), (128, H)).astype(
        ml_dtypes.bfloat16
    )

    iota = np.arange(32)[None, :] * 128 + np.arange(128)[:, None]  # [128, 32]

    in_maps = []
    for i in range(N_CORES):
        cfi = constsf.copy()
        cov2 = np.zeros((128, 4608), ml_dtypes.bfloat16)
        m = {"constsb": constsb, "constsf": cfi, "cov2": cov2}
        for j in range(nslots):
            b = int(assign[j, i])
            Lp = Lps[j]
            cfi[:, CF_MASK + j * 32 : CF_MASK + (j + 1) * 32] = (
                iota < int(lens[b])
            ).astype(np.float32)
            cov2[32 * j, 0:Lp] = coverage[b, 0, :Lp].astype(ml_dtypes.bfloat16)
            cov2[32 * j, Lp : Lp + H] = w1c_bf
            cov2[32 * j + 1, 0:Lp] = 1.0
            cov2[32 * j + 1, Lp : Lp + H] = biases[b].astype(ml_dtypes.bfloat16)
            tb = text_states[b, :Lp, :].astype(ml_dtypes.bfloat16)
            m[f"text{j}"] = tb
            m[f"ttex{j}"] = np.ascontiguousarray(tb.T)
        in_maps.append(m)

    return nc, in_maps, (assign, Lps, B, L)


def postprocess(outs, meta):
    assign, Lps, B, L = meta
    nslots = B // N_CORES
    context = np.zeros((B, FT), np.float32)
    attention = np.zeros((B, 1, L), np.float32)
    for i in range(N_CORES):
        for j in range(nslots):
            b = int(assign[j, i])
            context[b] = outs[i]["ctx"][j].astype(np.float32)
            attention[b, 0, : Lps[j]] = outs[i][f"att{j}"].reshape(-1)
    return context, attention


def kernel(text_states, summary_current_state, coverage, W1, b1, W2, b2, text_length):
    nc, in_maps, meta = prepare(
        text_states, summary_current_state, coverage, W1, b1, W2, b2, text_length
    )
    res = run_bass_kernel_spmd(nc, in_maps, list(range(N_CORES)))
    global LAST_RESULT
    LAST_RESULT = res
    return postprocess(res.results, meta)


# revision 38
# speedup vs baseline: 1.0185x; 1.0185x over previous
"""Trainium2 Bass kernel for ContextVectorNN (Bahdanau-style attention scorer).

Reference computation (per batch b):
  ts = text[b].T                              # [FT, L]
  x  = concat([ts, ss[b] bcast over L, cov])  # [C=1025, L]
  h  = tanh(W1 @ x + b1)                      # [H, L]
  logits = W2 @ h + b2                        # [1, L]
  att = softmax(mask(logits, len_b))          # [1, L]
  ctx = ts @ att.T                            # [FT]

Key structure used on device:
  - The summary channels are constant over L, so W1s @ ss[b] + b1 collapses to a
    per-batch bias vector (tiny; prepared host-side with the weights).
  - hT[l, h] = sum_c text[l, c] * W1T[c, h] via bf16 PE matmuls: transposed
    text tiles are the stationary operand, W1T chunks the moving operand.
  - coverage*w1c + bias are rank-1 in [L, H]: folded into the same PSUM
    accumulation as one extra K=2 matmul per L-tile, with the [cov; ones] rows
    and [w1c; bias] columns shipped inside the transposed-text input.
  - logits = sum_h W2[h]*tanh(hT[l, h]): tanh on ScalarE (PSUM -> SBUF bf16),
    then an in-place multiply by broadcast W2 and a free-axis reduce on DVE.
  - softmax uses a compile-time upper bound Bc >= max logit (exp never
    overflows), so no cross-partition max pass is needed; masked lanes are
    multiplied by 0 (exact zeros, matching softmax(-inf)). The denominator's
    cross-partition sum and broadcast are tiny PE matmuls with a ones vector.
  - attention beyond each batch's length is exactly 0, so each core only
    processes ceil(group_max_len/128)*128 positions per batch; batches are
    sorted by length and dealt one-per-core so the 8 cores stay balanced.

Sharding: data-parallel over batch across 8 cores (4 batches per core, one per
"slot"); weights replicated.

Implementation note: this walrus build allows very few semaphore waits per
instruction (1 for DMA/DVE ops, 2 for ScalarE), so the program is structured so
every instruction needs at most that: constants live in two packed arrays
observed once per engine at startup, hot buffers are manually multi-buffered
(no tile-pool slot releases), and cheap "observer" ops absorb cross-engine
dependencies before buffer reuse.
"""

import sys

sys.path.insert(0, "/opt/trn_rl_repo")

import numpy as np
import ml_dtypes

import concourse.bass as bass
import concourse.mybir as mybir
import concourse.tile as tile
from concourse.bass_utils import run_bass_kernel_spmd
from concourse.vector_clock import ScopedClock


def _spread_drain_and_barrier(self, tick_clock, wait_clock):
    """Replacement for TileContext._drain_and_barrier: this walrus build
    rejects instructions with more than one sync wait, and the kernel-tail
    drain normally carries the whole global clock. Spread those waits over
    individual one-wait NOPs on the sync engine first."""
    nc = self.nc
    probe = nc.sync.nop()
    wait_clock.add_sem_waits(probe.ins, ScopedClock({None: tick_clock.global_clock}))
    si = probe.ins.sync_info
    waits = list(si.on_wait or []) if si is not None else []
    if len(waits) > 1:
        probe.ins.sync_info = mybir.SyncInfo(
            on_wait=[waits[0]], on_update=list(si.on_update or [])
        )
        for w in waits[1:]:
            ex = nc.sync.nop()
            ex.ins.sync_info = mybir.SyncInfo(on_wait=[w], on_update=[])
    # SP executed the probe/extra NOPs in order, so the drain itself needs
    # no waits of its own.
    nc.sync.drain()
    nc.all_engine_barrier()
    assert self.sems is not None
    popped = nc._tile_sem_poison_stack.pop()
    assert popped is self._sem_poison
    nc.clear_and_free_semaphores(list(self.sems.allocated().values()))
    nc.all_engine_barrier()


tile.TileContext._drain_and_barrier = _spread_drain_and_barrier

BF16 = mybir.dt.bfloat16
F32 = mybir.dt.float32

N_CORES = 8
FT = 512
H = 512
KC = 4  # number of 128-channel chunks of the text features

# bf16 consts pack layout (columns): [w1tT chunks 0..3 | w2bc]
CB_W1T = 0
CB_W2 = KC * H
CB_COLS = KC * H + H
# f32 consts pack layout: [ones block 129 | ident 128 | mask 4*32 | bexp col]
CF_ONES = 0  # cols [0, 129) all ones: col 0 = ones col, row 0 = ones row
CF_IDENT = 129
CF_MASK = 129 + 128
CF_BEXP = 129 + 128 + 4 * 32
CF_COLS = 129 + 128 + 4 * 32 + 1


def build_program(Lps):
    """Build the SPMD Bass program. Lps: per-slot padded lengths (mult of 128)."""
    nslots = len(Lps)
    nc = bass.Bass()

    text_d = [
        nc.dram_tensor(f"text{j}", [Lps[j], FT], BF16, kind="ExternalInput")
        for j in range(nslots)
    ]
    # ttex: transposed text, rows = channels
    ttex_d = [
        nc.dram_tensor(f"ttex{j}", [512, Lps[j]], BF16, kind="ExternalInput")
        for j in range(nslots)
    ]
    # cov2: rows 32j   = [coverage_j | w1c], rows 32j+1 = [ones | bias_j]
    cov2_d = nc.dram_tensor("cov2", [128, 4608], BF16, kind="ExternalInput")
    cb_d = nc.dram_tensor("constsb", [128, CB_COLS], BF16, kind="ExternalInput")
    cf_d = nc.dram_tensor("constsf", [128, CF_COLS], F32, kind="ExternalInput")
    ctx_d = nc.dram_tensor("ctx", [nslots, FT], BF16, kind="ExternalOutput")
    att_d = [
        nc.dram_tensor(f"att{j}", [Lps[j] // 128, 128], F32, kind="ExternalOutput")
        for j in range(nslots)
    ]

    with tile.TileContext(nc) as tc:
        with (
            tc.tile_pool(name="consts", bufs=1) as consts,
            tc.tile_pool(name="data", bufs=1) as datap,
            tc.tile_pool(name="pfix", bufs=1, space="PSUM") as pfix,
        ):
            cb = consts.tile([128, CB_COLS], BF16)
            nc.sync.dma_start(out=cb[:], in_=cb_d[:])
            cf = consts.tile([128, CF_COLS], F32)
            nc.sync.dma_start(out=cf[:], in_=cf_d[:])
            cov2 = consts.tile([128, 4608], BF16)
            nc.sync.dma_start(out=cov2[:], in_=cov2_d[:])

            def w1t_sb(k):
                return cb[:, CB_W1T + k * H : CB_W1T + (k + 1) * H]

            w2bc_sb = cb[:, CB_W2 : CB_W2 + H]
            ones_col = cf[:, CF_ONES : CF_ONES + 1]
            ones_row = cf[0:1, CF_ONES + 1 : CF_ONES + 129]
            ident_sb = cf[:, CF_IDENT : CF_IDENT + 128]
            bexp_sb = cf[:, CF_BEXP : CF_BEXP + 1]

            def mask_sb(j, nt):
                return cf[:, CF_MASK + j * 32 : CF_MASK + j * 32 + nt]

            # fixed (manually rotated) buffers — avoids tile-pool release
            # semaphores, which would exceed per-instruction wait limits
            NH = 4
            htan = [consts.tile([128, H], BF16, name=f"ht{i}", tag=f"ht{i}") for i in range(NH)]
            NP = 3
            hps = [pfix.tile([128, H], F32, name=f"hp{i}", tag=f"hp{i}") for i in range(NP)]
            ctxps = [pfix.tile([1, FT], F32, name=f"cx{i}", tag=f"cx{i}") for i in range(2)]
            # shared small psum banks:
            # psA: [atT 0:128 | denom 128:160 | observers 162:166]
            psA = pfix.tile([128, 512], F32, tag="psA")
            psB = pfix.tile([128, 512], F32, tag="psB")  # denominator broadcast

            # engine observers: each engine waits once on the const DMAs so
            # later instructions elide those deps
            nc.tensor.matmul(
                psA[0:1, 162:163], cb[0:1, 0:1], cb[0:1, 0:1], start=True, stop=True
            )
            nc.tensor.matmul(
                psA[0:1, 163:164], cf[0:1, 0:1], cf[0:1, 0:1], start=True, stop=True
            )
            nc.tensor.matmul(
                psA[0:1, 166:167], cov2[0:1, 0:1], cov2[0:1, 0:1],
                start=True, stop=True,
            )
            scr_dve = consts.tile([1, 8], F32)
            nc.vector.tensor_copy(scr_dve[0:1, 0:1], cb[0:1, 0:1])
            nc.vector.tensor_copy(scr_dve[0:1, 1:2], cf[0:1, 0:1])
            scr_act = consts.tile([1, 128], F32)
            nc.scalar.activation(
                scr_act[0:1, 0:1], cf[0:1, 0:1], mybir.ActivationFunctionType.Tanh
            )
            nc.scalar.activation(
                scr_act[0:1, 1:2], cf[0:1, 0:1], mybir.ActivationFunctionType.Exp
            )

            ti = 0  # global L-tile counter for htan/hps rotation
            red_hist = []  # (logits tile, col) per completed DVE reduce
            pending_ctx = []  # deferred context-matmul emitters
            for j in range(nslots):
                nt = Lps[j] // 128
                Lp = Lps[j]

                # A) natural-layout text (context-matmul rhs); fresh tile
                nat = datap.tile(
                    [128, nt, FT], BF16, name=f"nat{j}", tag=f"nat{j}"
                )
                nc.sync.dma_start(
                    out=nat[:],
                    in_=text_d[j][:].rearrange("(t p) c -> p t c", p=128),
                )
                # B) transposed text chunks; fresh tile
                tsT = datap.tile(
                    [128, KC, Lp], BF16, name=f"tsT{j}", tag=f"tsT{j}"
                )
                nc.sync.dma_start(
                    out=tsT[:],
                    in_=ttex_d[j][:].rearrange("(k p) l -> p k l", p=128),
                )

                # PE observers for this slot's data DMAs
                nc.tensor.matmul(
                    psA[0:1, 164:165], nat[0:1, 0, 0:1], nat[0:1, 0, 0:1],
                    start=True, stop=True,
                )
                nc.tensor.matmul(
                    psA[0:1, 165:166], tsT[0:1, 0, 0:1], tsT[0:1, 0, 0:1],
                    start=True, stop=True,
                )

                logits = consts.tile([128, 32], F32, tag=f"lg{j}")

                if len(pending_ctx) > 1:
                    pending_ctx.pop(0)()

                for t in range(nt):
                    hp = hps[ti % NP]
                    ht = htan[ti % NH]
                    for k in range(KC):
                        nc.tensor.matmul(
                            hp[:],
                            tsT[:, k, t * 128 : (t + 1) * 128],
                            w1t_sb(k),
                            start=(k == 0),
                            stop=False,
                        )
                    nc.tensor.matmul(
                        hp[:],
                        cov2[32 * j : 32 * j + 2, t * 128 : (t + 1) * 128],
                        cov2[32 * j : 32 * j + 2, Lp : Lp + H],
                        start=False,
                        stop=True,
                        tile_position=(32 * j, 0),
                    )
                    if ti >= NH:
                        # make ACT observe the DVE tick that released ht:
                        # reading the reduce output of tile ti-2 (>= ti-NH)
                        plg, pt = red_hist[ti - 2]
                        col = 2 + (ti % 126)
                        nc.scalar.activation(
                            scr_act[0:1, col : col + 1],
                            plg[0:1, pt : pt + 1],
                            mybir.ActivationFunctionType.Copy,
                        )
                    nc.scalar.activation(
                        ht[:], hp[:], mybir.ActivationFunctionType.Tanh
                    )
                    # in-place: ht *= w2 (broadcast rows), then row-sum
                    nc.vector.tensor_tensor(
                        ht[:], ht[:], w2bc_sb, mybir.AluOpType.mult
                    )
                    nc.vector.tensor_reduce(
                        logits[:, t : t + 1],
                        ht[:],
                        axis=mybir.AxisListType.X,
                        op=mybir.AluOpType.add,
                    )
                    red_hist.append((logits, t))
                    ti += 1

                # ---- softmax tail (masked, bounded-exp)
                expv = consts.tile([128, 32], F32, tag=f"ex{j}")
                nc.scalar.activation(
                    expv[:, 0:nt],
                    logits[:, 0:nt],
                    mybir.ActivationFunctionType.Exp,
                    bias=bexp_sb,
                )
                attw = consts.tile([128, 32], F32, tag=f"aw{j}")
                nc.vector.tensor_tensor(
                    attw[:, 0:nt], expv[:, 0:nt], mask_sb(j, nt),
                    mybir.AluOpType.mult,
                )
                nc.tensor.matmul(
                    psA[0:1, 128 : 128 + nt], ones_col, attw[:, 0:nt],
                    start=True, stop=True,
                )
                den = consts.tile([1, 2], F32, tag=f"dn{j}")
                nc.vector.tensor_reduce(
                    den[0:1, 0:1], psA[0:1, 128 : 128 + nt],
                    axis=mybir.AxisListType.X, op=mybir.AluOpType.add,
                )
                nc.tensor.matmul(
                    psB[:, 0:1], ones_row, den[0:1, 0:1], start=True, stop=True
                )
                rcp = consts.tile([128, 2], F32, tag=f"rc{j}")
                nc.vector.reciprocal(rcp[:, 0:1], psB[:, 0:1])
                nc.vector.tensor_tensor(
                    attw[:, 0:nt],
                    attw[:, 0:nt],
                    rcp[:, 0:1].to_broadcast((128, nt)),
                    mybir.AluOpType.mult,
                )
                attb = consts.tile([128, 32], BF16, tag=f"ab{j}")
                nc.vector.tensor_copy(attb[:, 0:nt], attw[:, 0:nt])

                # attention out: PE-transpose [128, nt] -> [nt, 128], then DMA
                nc.tensor.matmul(
                    psA[0:nt, 0:128], attw[:, 0:nt], ident_sb,
                    is_transpose=True, start=True, stop=True,
                )
                atT = consts.tile([32, 128], F32, tag=f"at{j}")
                nc.vector.tensor_copy(atT[0:nt, :], psA[0:nt, 0:128])
                nc.gpsimd.dma_start(out=att_d[j][:], in_=atT[0:nt, :])

                # context: ctx[c] = sum_l att[l] * text[l, c].  Deferred so
                # the PE can start the next slot's h-matmuls while this
                # slot's softmax tail finishes on ACT/DVE.
                def emit_ctx(j=j, nt=nt, attb=attb, nat=nat):
                    cx = ctxps[j % 2]
                    for t in range(nt):
                        nc.tensor.matmul(
                            cx[:],
                            attb[:, t : t + 1],
                            nat[:, t, :],
                            start=(t == 0),
                            stop=(t == nt - 1),
                        )
                    cxs = consts.tile(
                        [1, FT], BF16, name=f"cs{j}", tag=f"cs{j}"
                    )
                    nc.vector.tensor_copy(cxs[:], cx[:])
                    nc.gpsimd.dma_start(out=ctx_d[j : j + 1, :], in_=cxs[:])

                pending_ctx.append(emit_ctx)

            for fn in pending_ctx:
                fn()

    return nc


def prepare(text_states, summary_current_state, coverage, W1, b1, W2, b2, text_length):
    B, L, ft = text_states.shape
    assert (ft, L) == (FT, 4096) and B == 32

    text_states = np.asarray(text_states, dtype=np.float32)
    summary_current_state = np.asarray(summary_current_state, dtype=np.float32)
    coverage = np.asarray(coverage, dtype=np.float32)
    W1 = np.asarray(W1, dtype=np.float32)
    b1 = np.asarray(b1, dtype=np.float32)
    W2 = np.asarray(W2, dtype=np.float32)
    b2 = np.asarray(b2, dtype=np.float32)
    lens = np.asarray(text_length).astype(np.int64)

    nslots = B // N_CORES

    # length-sorted assignment: slot j holds ranks [8j, 8j+8), one per core
    order = np.argsort(-lens, kind="stable")
    assign = order.reshape(nslots, N_CORES)  # assign[j, i] = batch of core i slot j
    Lps = [
        max(128, int(np.ceil(lens[assign[j]].max() / 128.0) * 128))
        for j in range(nslots)
    ]

    # softmax upper bound: |logit| <= sum|W2| with margin; b2 folded into exp
    Bc = float(np.abs(W2).sum()) + 1.0
    bexp = float(b2.reshape(-1)[0]) - Bc

    nc = build_program(Lps)

    # per-batch bias vectors (tiny): bias_b = W1s @ ss_b + b1
    W1t = W1[:, :FT]
    W1s = W1[:, FT : FT + FT]
    w1c = W1[:, -1]
    biases = summary_current_state @ W1s.T + b1[None, :]  # [B, H] fp32
    w1c_bf = w1c.astype(ml_dtypes.bfloat16)

    constsf = np.zeros((128, CF_COLS), np.float32)
    constsf[:, CF_ONES : CF_ONES + 129] = 1.0
    constsf[:, CF_IDENT : CF_IDENT + 128] = np.eye(128, dtype=np.float32)
    constsf[:, CF_BEXP] = bexp

    constsb = np.zeros((128, CB_COLS), ml_dtypes.bfloat16)
    constsb[:, CB_W1T : CB_W1T + KC * H] = (
        np.ascontiguousarray(W1t.T)
        .reshape(KC, 128, H)
        .transpose(1, 0, 2)
        .reshape(128, KC * H)
        .astype(ml_dtypes.bfloat16)
    )
    constsb[:, CB_W2 : CB_W2 + H] = np.broadcast_to(W2.reshape(1, H), (128, H)).astype(
        ml_dtypes.bfloat16
    )

    iota = np.arange(32)[None, :] * 128 + np.arange(128)[:, None]  # [128, 32]

    in_maps = []
    for i in range(N_CORES):
        cfi = constsf.copy()
        cov2 = np.zeros((128, 4608), ml_dtypes.bfloat16)
        m = {"constsb": constsb, "constsf": cfi, "cov2": cov2}
        for j in range(nslots):
            b = int(assign[j, i])
            Lp = Lps[j]
            cfi[:, CF_MASK + j * 32 : CF_MASK + (j + 1) * 32] = (
                iota < int(lens[b])
            ).astype(np.float32)
            cov2[32 * j, 0:Lp] = coverage[b, 0, :Lp].astype(ml_dtypes.bfloat16)
            cov2[32 * j, Lp : Lp + H] = w1c_bf
            cov2[32 * j + 1, 0:Lp] = 1.0
            cov2[32 * j + 1, Lp : Lp + H] = biases[b].astype(ml_dtypes.bfloat16)
            tb = text_states[b, :Lp, :].astype(ml_dtypes.bfloat16)
            m[f"text{j}"] = tb
            m[f"ttex{j}"] = np.ascontiguousarray(tb.T)
        in_maps.append(m)

    return nc, in_maps, (assign, Lps, B, L)


def postprocess(outs, meta):
    assign, Lps, B, L = meta
    nslots = B // N_CORES
    context = np.zeros((B, FT), np.float32)
    attention = np.zeros((B, 1, L), np.float32)
    for i in range(N_CORES):
        for j in range(nslots):
            b = int(assign[j, i])
            context[b] = outs[i]["ctx"][j].astype(np.float32)
            attention[b, 0, : Lps[j]] = outs[i][f"att{j}"].reshape(-1)
    return context, attention


def kernel(text_states, summary_current_state, coverage, W1, b1, W2, b2, text_length):
    nc, in_maps, meta = prepare(
        text_states, summary_current_state, coverage, W1, b1, W2, b2, text_length
    )
    res = run_bass_kernel_spmd(nc, in_maps, list(range(N_CORES)))
    global LAST_RESULT
    LAST_RESULT = res
    return postprocess(res.results, meta)


# revision 41
# speedup vs baseline: 1.2070x; 1.1851x over previous
"""Trainium2 Bass kernel for ContextVectorNN (Bahdanau-style attention scorer).

Reference computation (per batch b):
  ts = text[b].T                              # [FT, L]
  x  = concat([ts, ss[b] bcast over L, cov])  # [C=1025, L]
  h  = tanh(W1 @ x + b1)                      # [H, L]
  logits = W2 @ h + b2                        # [1, L]
  att = softmax(mask(logits, len_b))          # [1, L]
  ctx = ts @ att.T                            # [FT]

Key structure used on device:
  - The summary channels are constant over L, so W1s @ ss[b] + b1 collapses to a
    per-batch bias vector (tiny; prepared host-side with the weights).
  - hT[l, h] = sum_c text[l, c] * W1T[c, h] via bf16 PE matmuls: transposed
    text tiles are the stationary operand, W1T chunks the moving operand.
  - coverage*w1c + bias are rank-1 in [L, H]: folded into the same PSUM
    accumulation as one extra K=2 matmul per L-tile, with the [cov; ones] rows
    and [w1c; bias] columns shipped inside the transposed-text input.
  - logits = sum_h W2[h]*tanh(hT[l, h]): tanh on ScalarE (PSUM -> SBUF bf16),
    then an in-place multiply by broadcast W2 and a free-axis reduce on DVE.
  - softmax uses a compile-time upper bound Bc >= max logit (exp never
    overflows), so no cross-partition max pass is needed; masked lanes are
    multiplied by 0 (exact zeros, matching softmax(-inf)). The denominator's
    cross-partition sum and broadcast are tiny PE matmuls with a ones vector.
  - attention beyond each batch's length is exactly 0, so each core only
    processes ceil(group_max_len/128)*128 positions per batch; batches are
    sorted by length and dealt one-per-core so the 8 cores stay balanced.

Sharding: data-parallel over batch across 8 cores (4 batches per core, one per
"slot"); weights replicated.

Implementation note: this walrus build allows very few semaphore waits per
instruction (1 for DMA/DVE ops, 2 for ScalarE), so the program is structured so
every instruction needs at most that: constants live in two packed arrays
observed once per engine at startup, hot buffers are manually multi-buffered
(no tile-pool slot releases), and cheap "observer" ops absorb cross-engine
dependencies before buffer reuse.
"""

import sys

sys.path.insert(0, "/opt/trn_rl_repo")

import numpy as np
import ml_dtypes

import concourse.bass as bass
import concourse.mybir as mybir
import concourse.tile as tile
from concourse.bass_utils import run_bass_kernel_spmd
from concourse.vector_clock import ScopedClock


def _spread_drain_and_barrier(self, tick_clock, wait_clock):
    """Replacement for TileContext._drain_and_barrier: this walrus build
    rejects instructions with more than one sync wait, and the kernel-tail
    drain normally carries the whole global clock. Spread those waits over
    individual one-wait NOPs on the sync engine first."""
    nc = self.nc
    probe = nc.sync.nop()
    wait_clock.add_sem_waits(probe.ins, ScopedClock({None: tick_clock.global_clock}))
    si = probe.ins.sync_info
    waits = list(si.on_wait or []) if si is not None else []
    if len(waits) > 1:
        probe.ins.sync_info = mybir.SyncInfo(
            on_wait=[waits[0]], on_update=list(si.on_update or [])
        )
        for w in waits[1:]:
            ex = nc.sync.nop()
            ex.ins.sync_info = mybir.SyncInfo(on_wait=[w], on_update=[])
    # SP executed the probe/extra NOPs in order, so the drain itself needs
    # no waits of its own.
    nc.sync.drain()
    nc.all_engine_barrier()
    assert self.sems is not None
    popped = nc._tile_sem_poison_stack.pop()
    assert popped is self._sem_poison
    nc.clear_and_free_semaphores(list(self.sems.allocated().values()))
    nc.all_engine_barrier()


tile.TileContext._drain_and_barrier = _spread_drain_and_barrier

BF16 = mybir.dt.bfloat16
F32 = mybir.dt.float32

N_CORES = 8
FT = 512
H = 512
KC = 4  # number of 128-channel chunks of the text features

# bf16 consts pack layout (columns): [w1tT chunks 0..3 | w2bc]
CB_W1T = 0
CB_W2 = KC * H
CB_COLS = KC * H + H
# f32 consts pack layout: [ones block 129 | ident 128 | mask 4*32 | bexp col]
CF_ONES = 0  # cols [0, 129) all ones: col 0 = ones col, row 0 = ones row
CF_IDENT = 129
CF_MASK = 129 + 128
CF_BEXP = 129 + 128 + 4 * 32
CF_COLS = 129 + 128 + 4 * 32 + 1


def build_program(Lps):
    """Build the SPMD Bass program. Lps: per-slot padded lengths (mult of 128)."""
    nslots = len(Lps)
    nc = bass.Bass()

    text_d = [
        nc.dram_tensor(f"text{j}", [Lps[j], FT], BF16, kind="ExternalInput")
        for j in range(nslots)
    ]
    # ttex: transposed text, rows = channels
    ttex_d = [
        nc.dram_tensor(f"ttex{j}", [512, Lps[j]], BF16, kind="ExternalInput")
        for j in range(nslots)
    ]
    # cov2: rows 32j   = [coverage_j | w1c], rows 32j+1 = [ones | bias_j]
    cov2_d = nc.dram_tensor("cov2", [128, 4608], BF16, kind="ExternalInput")
    cb_d = nc.dram_tensor("constsb", [128, CB_COLS], BF16, kind="ExternalInput")
    cf_d = nc.dram_tensor("constsf", [128, CF_COLS], F32, kind="ExternalInput")
    ctx_d = nc.dram_tensor("ctx", [nslots, FT], BF16, kind="ExternalOutput")
    att_d = [
        nc.dram_tensor(f"att{j}", [Lps[j] // 128, 128], F32, kind="ExternalOutput")
        for j in range(nslots)
    ]

    with tile.TileContext(nc) as tc:
        with (
            tc.tile_pool(name="consts", bufs=1) as consts,
            tc.tile_pool(name="data", bufs=1) as datap,
            tc.tile_pool(name="pfix", bufs=1, space="PSUM") as pfix,
        ):
            cb = consts.tile([128, CB_COLS], BF16)
            nc.sync.dma_start(out=cb[:], in_=cb_d[:])
            cf = consts.tile([128, CF_COLS], F32)
            nc.sync.dma_start(out=cf[:], in_=cf_d[:])
            cov2 = consts.tile([128, 4608], BF16)
            nc.sync.dma_start(out=cov2[:], in_=cov2_d[:])

            def w1t_sb(k):
                return cb[:, CB_W1T + k * H : CB_W1T + (k + 1) * H]

            w2bc_sb = cb[:, CB_W2 : CB_W2 + H]
            ones_col = cf[:, CF_ONES : CF_ONES + 1]
            ones_row = cf[0:1, CF_ONES + 1 : CF_ONES + 129]
            ident_sb = cf[:, CF_IDENT : CF_IDENT + 128]
            bexp_sb = cf[:, CF_BEXP : CF_BEXP + 1]

            def mask_sb(j, nt):
                return cf[:, CF_MASK + j * 32 : CF_MASK + j * 32 + nt]

            # fixed (manually rotated) buffers — avoids tile-pool release
            # semaphores, which would exceed per-instruction wait limits
            NH = 4
            htan = [consts.tile([128, H], BF16, name=f"ht{i}", tag=f"ht{i}") for i in range(NH)]
            NP = 3
            hps = [pfix.tile([128, H], F32, name=f"hp{i}", tag=f"hp{i}") for i in range(NP)]
            ctxps = [pfix.tile([1, FT], F32, name=f"cx{i}", tag=f"cx{i}") for i in range(2)]
            # shared small psum banks:
            # psA: [atT 0:128 | denom 128:160 | observers 162:166]
            psA = pfix.tile([128, 512], F32, tag="psA")
            psB = pfix.tile([128, 512], F32, tag="psB")  # denominator broadcast

            # engine observers: each engine waits once on the const DMAs so
            # later instructions elide those deps
            nc.tensor.matmul(
                psA[0:1, 162:163], cb[0:1, 0:1], cb[0:1, 0:1], start=True, stop=True
            )
            nc.tensor.matmul(
                psA[0:1, 163:164], cf[0:1, 0:1], cf[0:1, 0:1], start=True, stop=True
            )
            nc.tensor.matmul(
                psA[0:1, 166:167], cov2[0:1, 0:1], cov2[0:1, 0:1],
                start=True, stop=True,
            )
            scr_dve = consts.tile([1, 8], F32)
            nc.vector.tensor_copy(scr_dve[0:1, 0:1], cb[0:1, 0:1])
            nc.vector.tensor_copy(scr_dve[0:1, 1:2], cf[0:1, 0:1])
            scr_act = consts.tile([1, 128], F32)
            nc.scalar.activation(
                scr_act[0:1, 0:1], cf[0:1, 0:1], mybir.ActivationFunctionType.Tanh
            )
            nc.scalar.activation(
                scr_act[0:1, 1:2], cf[0:1, 0:1], mybir.ActivationFunctionType.Exp
            )

            ti = 0  # global L-tile counter for htan/hps rotation
            red_hist = []  # (logits tile, col) per completed DVE reduce
            pending_ctx = []  # deferred context-matmul emitters
            for j in range(nslots):
                nt = Lps[j] // 128
                Lp = Lps[j]

                # A) transposed text chunks (the critical-path operand),
                # loaded in column chunks so the first h-matmuls can start
                # before the whole slot has landed
                tsT = datap.tile(
                    [128, KC, Lp], BF16, name=f"tsT{j}", tag=f"tsT{j}"
                )
                nch = max(1, nt // 8)
                cw = (nt + nch - 1) // nch * 128
                chunk_starts = list(range(0, Lp, cw))
                for c0 in chunk_starts:
                    c1 = min(Lp, c0 + cw)
                    nc.sync.dma_start(
                        out=tsT[:, :, c0:c1],
                        in_=ttex_d[j][:, c0:c1].rearrange(
                            "(k p) l -> p k l", p=128
                        ),
                    )
                # B) natural-layout text (context-matmul rhs): only needed at
                # the end of the slot, so it loads behind the tsT chunks
                nat = datap.tile(
                    [128, nt, FT], BF16, name=f"nat{j}", tag=f"nat{j}"
                )
                nc.sync.dma_start(
                    out=nat[:],
                    in_=text_d[j][:].rearrange("(t p) c -> p t c", p=128),
                )

                logits = consts.tile([128, 32], F32, tag=f"lg{j}")

                if len(pending_ctx) > 1:
                    pending_ctx.pop(0)()

                for t in range(nt):
                    if t * 128 in chunk_starts:
                        # PE observer for this tsT column chunk's DMA
                        oc = 164 + (j * 8 + (t * 128) // cw) % 340
                        nc.tensor.matmul(
                            psA[0:1, oc : oc + 1],
                            tsT[0:1, 0, t * 128 : t * 128 + 1],
                            tsT[0:1, 0, t * 128 : t * 128 + 1],
                            start=True, stop=True,
                        )
                    hp = hps[ti % NP]
                    ht = htan[ti % NH]
                    for k in range(KC):
                        nc.tensor.matmul(
                            hp[:],
                            tsT[:, k, t * 128 : (t + 1) * 128],
                            w1t_sb(k),
                            start=(k == 0),
                            stop=False,
                        )
                    nc.tensor.matmul(
                        hp[:],
                        cov2[32 * j : 32 * j + 2, t * 128 : (t + 1) * 128],
                        cov2[32 * j : 32 * j + 2, Lp : Lp + H],
                        start=False,
                        stop=True,
                        tile_position=(32 * j, 0),
                    )
                    if ti >= NH and ti % 2 == 0:
                        # make ACT observe the DVE tick that released the next
                        # 2 ht buffers: reduce(ti-2) >= reduce(ti-NH+1)
                        plg, pt = red_hist[ti - 2]
                        col = 2 + (ti % 126)
                        nc.scalar.activation(
                            scr_act[0:1, col : col + 1],
                            plg[0:1, pt : pt + 1],
                            mybir.ActivationFunctionType.Copy,
                        )
                    nc.scalar.activation(
                        ht[:], hp[:], mybir.ActivationFunctionType.Tanh
                    )
                    # in-place: ht *= w2 (broadcast rows), then row-sum
                    nc.vector.tensor_tensor(
                        ht[:], ht[:], w2bc_sb, mybir.AluOpType.mult
                    )
                    nc.vector.tensor_reduce(
                        logits[:, t : t + 1],
                        ht[:],
                        axis=mybir.AxisListType.X,
                        op=mybir.AluOpType.add,
                    )
                    red_hist.append((logits, t))
                    ti += 1

                # ---- softmax tail (masked, bounded-exp)
                expv = consts.tile([128, 32], F32, tag=f"ex{j}")
                nc.scalar.activation(
                    expv[:, 0:nt],
                    logits[:, 0:nt],
                    mybir.ActivationFunctionType.Exp,
                    bias=bexp_sb,
                )
                attw = consts.tile([128, 32], F32, tag=f"aw{j}")
                nc.vector.tensor_tensor(
                    attw[:, 0:nt], expv[:, 0:nt], mask_sb(j, nt),
                    mybir.AluOpType.mult,
                )
                nc.tensor.matmul(
                    psA[0:1, 128 : 128 + nt], ones_col, attw[:, 0:nt],
                    start=True, stop=True,
                )
                den = consts.tile([1, 2], F32, tag=f"dn{j}")
                nc.vector.tensor_reduce(
                    den[0:1, 0:1], psA[0:1, 128 : 128 + nt],
                    axis=mybir.AxisListType.X, op=mybir.AluOpType.add,
                )
                nc.tensor.matmul(
                    psB[:, 0:1], ones_row, den[0:1, 0:1], start=True, stop=True
                )
                rcp = consts.tile([128, 2], F32, tag=f"rc{j}")
                nc.vector.reciprocal(rcp[:, 0:1], psB[:, 0:1])
                nc.vector.tensor_tensor(
                    attw[:, 0:nt],
                    attw[:, 0:nt],
                    rcp[:, 0:1].to_broadcast((128, nt)),
                    mybir.AluOpType.mult,
                )
                attb = consts.tile([128, 32], BF16, tag=f"ab{j}")
                nc.vector.tensor_copy(attb[:, 0:nt], attw[:, 0:nt])

                # attention out: PE-transpose [128, nt] -> [nt, 128], then DMA
                nc.tensor.matmul(
                    psA[0:nt, 0:128], attw[:, 0:nt], ident_sb,
                    is_transpose=True, start=True, stop=True,
                )
                atT = consts.tile([32, 128], F32, tag=f"at{j}")
                nc.vector.tensor_copy(atT[0:nt, :], psA[0:nt, 0:128])
                nc.gpsimd.dma_start(out=att_d[j][:], in_=atT[0:nt, :])

                # context: ctx[c] = sum_l att[l] * text[l, c].  Deferred so
                # the PE can start the next slot's h-matmuls while this
                # slot's softmax tail finishes on ACT/DVE.
                def emit_ctx(j=j, nt=nt, attb=attb, nat=nat):
                    oc = 504 + j
                    nc.tensor.matmul(
                        psA[0:1, oc : oc + 1], nat[0:1, 0, 0:1], nat[0:1, 0, 0:1],
                        start=True, stop=True,
                    )
                    cx = ctxps[j % 2]
                    for t in range(nt):
                        nc.tensor.matmul(
                            cx[:],
                            attb[:, t : t + 1],
                            nat[:, t, :],
                            start=(t == 0),
                            stop=(t == nt - 1),
                        )
                    cxs = consts.tile(
                        [1, FT], BF16, name=f"cs{j}", tag=f"cs{j}"
                    )
                    nc.vector.tensor_copy(cxs[:], cx[:])
                    nc.gpsimd.dma_start(out=ctx_d[j : j + 1, :], in_=cxs[:])

                pending_ctx.append(emit_ctx)

            for fn in pending_ctx:
                fn()

    return nc


def prepare(text_states, summary_current_state, coverage, W1, b1, W2, b2, text_length):
    B, L, ft = text_states.shape
    assert (ft, L) == (FT, 4096) and B == 32

    text_states = np.asarray(text_states, dtype=np.float32)
    summary_current_state = np.asarray(summary_current_state, dtype=np.float32)
    coverage = np.asarray(coverage, dtype=np.float32)
    W1 = np.asarray(W1, dtype=np.float32)
    b1 = np.asarray(b1, dtype=np.float32)
    W2 = np.asarray(W2, dtype=np.float32)
    b2 = np.asarray(b2, dtype=np.float32)
    lens = np.asarray(text_length).astype(np.int64)

    nslots = B // N_CORES

    # length-sorted assignment: slot j holds ranks [8j, 8j+8), one per core
    order = np.argsort(-lens, kind="stable")
    assign = order.reshape(nslots, N_CORES)  # assign[j, i] = batch of core i slot j
    Lps = [
        max(128, int(np.ceil(lens[assign[j]].max() / 128.0) * 128))
        for j in range(nslots)
    ]

    # softmax upper bound: |logit| <= sum|W2| with margin; b2 folded into exp
    Bc = float(np.abs(W2).sum()) + 1.0
    bexp = float(b2.reshape(-1)[0]) - Bc

    nc = build_program(Lps)

    # per-batch bias vectors (tiny): bias_b = W1s @ ss_b + b1
    W1t = W1[:, :FT]
    W1s = W1[:, FT : FT + FT]
    w1c = W1[:, -1]
    biases = summary_current_state @ W1s.T + b1[None, :]  # [B, H] fp32
    w1c_bf = w1c.astype(ml_dtypes.bfloat16)

    constsf = np.zeros((128, CF_COLS), np.float32)
    constsf[:, CF_ONES : CF_ONES + 129] = 1.0
    constsf[:, CF_IDENT : CF_IDENT + 128] = np.eye(128, dtype=np.float32)
    constsf[:, CF_BEXP] = bexp

    constsb = np.zeros((128, CB_COLS), ml_dtypes.bfloat16)
    constsb[:, CB_W1T : CB_W1T + KC * H] = (
        np.ascontiguousarray(W1t.T)
        .reshape(KC, 128, H)
        .transpose(1, 0, 2)
        .reshape(128, KC * H)
        .astype(ml_dtypes.bfloat16)
    )
    constsb[:, CB_W2 : CB_W2 + H] = np.broadcast_to(W2.reshape(1, H), (128, H)).astype(
        ml_dtypes.bfloat16
    )

    iota = np.arange(32)[None, :] * 128 + np.arange(128)[:, None]  # [128, 32]

    in_maps = []
    for i in range(N_CORES):
        cfi = constsf.copy()
        cov2 = np.zeros((128, 4608), ml_dtypes.bfloat16)
        m = {"constsb": constsb, "constsf": cfi, "cov2": cov2}
        for j in range(nslots):
            b = int(assign[j, i])
            Lp = Lps[j]
            cfi[:, CF_MASK + j * 32 : CF_MASK + (j + 1) * 32] = (
                iota < int(lens[b])
            ).astype(np.float32)
            cov2[32 * j, 0:Lp] = coverage[b, 0, :Lp].astype(ml_dtypes.bfloat16)
            cov2[32 * j, Lp : Lp + H] = w1c_bf
            cov2[32 * j + 1, 0:Lp] = 1.0
            cov2[32 * j + 1, Lp : Lp + H] = biases[b].astype(ml_dtypes.bfloat16)
            tb = text_states[b, :Lp, :].astype(ml_dtypes.bfloat16)
            m[f"text{j}"] = tb
            m[f"ttex{j}"] = np.ascontiguousarray(tb.T)
        in_maps.append(m)

    return nc, in_maps, (assign, Lps, B, L)


def postprocess(outs, meta):
    assign, Lps, B, L = meta
    nslots = B // N_CORES
    context = np.zeros((B, FT), np.float32)
    attention = np.zeros((B, 1, L), np.float32)
    for i in range(N_CORES):
        for j in range(nslots):
            b = int(assign[j, i])
            context[b] = outs[i]["ctx"][j].astype(np.float32)
            attention[b, 0, : Lps[j]] = outs[i][f"att{j}"].reshape(-1)
    return context, attention


def kernel(text_states, summary_current_state, coverage, W1, b1, W2, b2, text_length):
    nc, in_maps, meta = prepare(
        text_states, summary_current_state, coverage, W1, b1, W2, b2, text_length
    )
    res = run_bass_kernel_spmd(nc, in_maps, list(range(N_CORES)))
    global LAST_RESULT
    LAST_RESULT = res
    return postprocess(res.results, meta)


# revision 43
# speedup vs baseline: 1.2192x; 1.0101x over previous
"""Trainium2 Bass kernel for ContextVectorNN (Bahdanau-style attention scorer).

Reference computation (per batch b):
  ts = text[b].T                              # [FT, L]
  x  = concat([ts, ss[b] bcast over L, cov])  # [C=1025, L]
  h  = tanh(W1 @ x + b1)                      # [H, L]
  logits = W2 @ h + b2                        # [1, L]
  att = softmax(mask(logits, len_b))          # [1, L]
  ctx = ts @ att.T                            # [FT]

Key structure used on device:
  - The summary channels are constant over L, so W1s @ ss[b] + b1 collapses to a
    per-batch bias vector (tiny; prepared host-side with the weights).
  - hT[l, h] = sum_c text[l, c] * W1T[c, h] via bf16 PE matmuls: transposed
    text tiles are the stationary operand, W1T chunks the moving operand.
  - coverage*w1c + bias are rank-1 in [L, H]: folded into the same PSUM
    accumulation as one extra K=2 matmul per L-tile, with the [cov; ones] rows
    and [w1c; bias] columns shipped inside the transposed-text input.
  - logits = sum_h W2[h]*tanh(hT[l, h]): tanh on ScalarE (PSUM -> SBUF bf16),
    then an in-place multiply by broadcast W2 and a free-axis reduce on DVE.
  - softmax uses a compile-time upper bound Bc >= max logit (exp never
    overflows), so no cross-partition max pass is needed; masked lanes are
    multiplied by 0 (exact zeros, matching softmax(-inf)). The denominator's
    cross-partition sum and broadcast are tiny PE matmuls with a ones vector.
  - attention beyond each batch's length is exactly 0, so each core only
    processes ceil(group_max_len/128)*128 positions per batch; batches are
    sorted by length and dealt one-per-core so the 8 cores stay balanced.

Sharding: data-parallel over batch across 8 cores (4 batches per core, one per
"slot"); weights replicated.

Implementation note: this walrus build allows very few semaphore waits per
instruction (1 for DMA/DVE ops, 2 for ScalarE), so the program is structured so
every instruction needs at most that: constants live in two packed arrays
observed once per engine at startup, hot buffers are manually multi-buffered
(no tile-pool slot releases), and cheap "observer" ops absorb cross-engine
dependencies before buffer reuse.
"""

import sys

sys.path.insert(0, "/opt/trn_rl_repo")

import numpy as np
import ml_dtypes

import concourse.bass as bass
import concourse.mybir as mybir
import concourse.tile as tile
from concourse.bass_utils import run_bass_kernel_spmd
from concourse.vector_clock import ScopedClock


def _spread_drain_and_barrier(self, tick_clock, wait_clock):
    """Replacement for TileContext._drain_and_barrier: this walrus build
    rejects instructions with more than one sync wait, and the kernel-tail
    drain normally carries the whole global clock. Spread those waits over
    individual one-wait NOPs on the sync engine first."""
    nc = self.nc
    probe = nc.sync.nop()
    wait_clock.add_sem_waits(probe.ins, ScopedClock({None: tick_clock.global_clock}))
    si = probe.ins.sync_info
    waits = list(si.on_wait or []) if si is not None else []
    if len(waits) > 1:
        probe.ins.sync_info = mybir.SyncInfo(
            on_wait=[waits[0]], on_update=list(si.on_update or [])
        )
        for w in waits[1:]:
            ex = nc.sync.nop()
            ex.ins.sync_info = mybir.SyncInfo(on_wait=[w], on_update=[])
    # SP executed the probe/extra NOPs in order, so the drain itself needs
    # no waits of its own.
    nc.sync.drain()
    nc.all_engine_barrier()
    assert self.sems is not None
    popped = nc._tile_sem_poison_stack.pop()
    assert popped is self._sem_poison
    nc.clear_and_free_semaphores(list(self.sems.allocated().values()))
    nc.all_engine_barrier()


tile.TileContext._drain_and_barrier = _spread_drain_and_barrier

BF16 = mybir.dt.bfloat16
F32 = mybir.dt.float32

N_CORES = 8
FT = 512
H = 512
KC = 4  # number of 128-channel chunks of the text features

# bf16 consts pack layout (columns): [w1tT chunks 0..3 | w2bc]
CB_W1T = 0
CB_W2 = KC * H
CB_COLS = KC * H + H
# f32 consts pack layout: [ones block 129 | ident 128 | mask 4*32 | bexp col]
CF_ONES = 0  # cols [0, 129) all ones: col 0 = ones col, row 0 = ones row
CF_IDENT = 129
CF_MASK = 129 + 128
CF_BEXP = 129 + 128 + 4 * 32
CF_COLS = 129 + 128 + 4 * 32 + 1


def build_program(Lps):
    """Build the SPMD Bass program. Lps: per-slot padded lengths (mult of 128)."""
    nslots = len(Lps)
    nc = bass.Bass()

    text_d = [
        nc.dram_tensor(f"text{j}", [Lps[j], FT], BF16, kind="ExternalInput")
        for j in range(nslots)
    ]
    # ttex: transposed text, rows = channels
    ttex_d = [
        nc.dram_tensor(f"ttex{j}", [512, Lps[j]], BF16, kind="ExternalInput")
        for j in range(nslots)
    ]
    # cov2: rows 32j   = [coverage_j | w1c], rows 32j+1 = [ones | bias_j]
    cov2_d = nc.dram_tensor("cov2", [128, 4608], BF16, kind="ExternalInput")
    cb_d = nc.dram_tensor("constsb", [128, CB_COLS], BF16, kind="ExternalInput")
    cf_d = nc.dram_tensor("constsf", [128, CF_COLS], F32, kind="ExternalInput")
    ctx_d = nc.dram_tensor("ctx", [nslots, FT], BF16, kind="ExternalOutput")
    att_d = [
        nc.dram_tensor(f"att{j}", [Lps[j] // 128, 128], F32, kind="ExternalOutput")
        for j in range(nslots)
    ]

    with tile.TileContext(nc) as tc:
        with (
            tc.tile_pool(name="consts", bufs=1) as consts,
            tc.tile_pool(name="data", bufs=1) as datap,
            tc.tile_pool(name="pfix", bufs=1, space="PSUM") as pfix,
        ):
            cb = consts.tile([128, CB_COLS], BF16)
            nc.sync.dma_start(out=cb[:], in_=cb_d[:])
            cf = consts.tile([128, CF_COLS], F32)
            nc.sync.dma_start(out=cf[:], in_=cf_d[:])
            cov2 = consts.tile([128, 4608], BF16)
            nc.sync.dma_start(out=cov2[:], in_=cov2_d[:])

            def w1t_sb(k):
                return cb[:, CB_W1T + k * H : CB_W1T + (k + 1) * H]

            w2bc_sb = cb[:, CB_W2 : CB_W2 + H]
            ones_col = cf[:, CF_ONES : CF_ONES + 1]
            ones_row = cf[0:1, CF_ONES + 1 : CF_ONES + 129]
            ident_sb = cf[:, CF_IDENT : CF_IDENT + 128]
            bexp_sb = cf[:, CF_BEXP : CF_BEXP + 1]

            def mask_sb(j, nt):
                return cf[:, CF_MASK + j * 32 : CF_MASK + j * 32 + nt]

            # fixed (manually rotated) buffers — avoids tile-pool release
            # semaphores, which would exceed per-instruction wait limits
            NH = 4
            htan = [consts.tile([128, H], BF16, name=f"ht{i}", tag=f"ht{i}") for i in range(NH)]
            NP = 4
            hps = [pfix.tile([128, H], F32, name=f"hp{i}", tag=f"hp{i}") for i in range(NP)]
            ctxps = [pfix.tile([1, FT], F32, name=f"cx{i}", tag=f"cx{i}") for i in range(2)]
            # shared small psum banks:
            # psA: [atT 0:128 | denom 128:160 | observers 162:166]
            psA = pfix.tile([128, 512], F32, tag="psA")
            psB = pfix.tile([128, 512], F32, tag="psB")  # denominator broadcast

            # engine observers: each engine waits once on the const DMAs so
            # later instructions elide those deps
            nc.tensor.matmul(
                psA[0:1, 162:163], cb[0:1, 0:1], cb[0:1, 0:1], start=True, stop=True
            )
            nc.tensor.matmul(
                psA[0:1, 163:164], cf[0:1, 0:1], cf[0:1, 0:1], start=True, stop=True
            )
            nc.tensor.matmul(
                psA[0:1, 166:167], cov2[0:1, 0:1], cov2[0:1, 0:1],
                start=True, stop=True,
            )
            scr_dve = consts.tile([1, 8], F32)
            nc.vector.tensor_copy(scr_dve[0:1, 0:1], cb[0:1, 0:1])
            nc.vector.tensor_copy(scr_dve[0:1, 1:2], cf[0:1, 0:1])
            scr_act = consts.tile([1, 128], F32)
            nc.scalar.activation(
                scr_act[0:1, 0:1], cf[0:1, 0:1], mybir.ActivationFunctionType.Tanh
            )
            nc.scalar.activation(
                scr_act[0:1, 1:2], cf[0:1, 0:1], mybir.ActivationFunctionType.Exp
            )

            ti = 0  # global L-tile counter for htan/hps rotation
            red_hist = []  # (logits tile, col) per completed DVE reduce
            pending_ctx = []  # deferred context-matmul emitters
            for j in range(nslots):
                nt = Lps[j] // 128
                Lp = Lps[j]

                # A) transposed text chunks (the critical-path operand),
                # loaded in column chunks so the first h-matmuls can start
                # before the whole slot has landed
                tsT = datap.tile(
                    [128, KC, Lp], BF16, name=f"tsT{j}", tag=f"tsT{j}"
                )
                # progressive chunks: a tiny first chunk lets the PE start
                # almost immediately; larger chunks amortize DMA overhead
                chunk_starts = []
                c0 = 0
                step = 2 * 128 if j == 0 else 8 * 128
                while c0 < Lp:
                    chunk_starts.append(c0)
                    c1 = min(Lp, c0 + step)
                    nc.sync.dma_start(
                        out=tsT[:, :, c0:c1],
                        in_=ttex_d[j][:, c0:c1].rearrange(
                            "(k p) l -> p k l", p=128
                        ),
                    )
                    c0 = c1
                    step = min(8 * 128, step * 2)
                # B) natural-layout text (context-matmul rhs): only needed at
                # the end of the slot, so it loads behind the tsT chunks
                nat = datap.tile(
                    [128, nt, FT], BF16, name=f"nat{j}", tag=f"nat{j}"
                )
                nc.sync.dma_start(
                    out=nat[:],
                    in_=text_d[j][:].rearrange("(t p) c -> p t c", p=128),
                )

                logits = consts.tile([128, 32], F32, tag=f"lg{j}")

                if len(pending_ctx) > 1:
                    pending_ctx.pop(0)()

                for t in range(nt):
                    if t * 128 in chunk_starts:
                        # PE observer for this tsT column chunk's DMA
                        oc = 164 + (j * 16 + chunk_starts.index(t * 128)) % 336
                        nc.tensor.matmul(
                            psA[0:1, oc : oc + 1],
                            tsT[0:1, 0, t * 128 : t * 128 + 1],
                            tsT[0:1, 0, t * 128 : t * 128 + 1],
                            start=True, stop=True,
                        )
                    hp = hps[ti % NP]
                    ht = htan[ti % NH]
                    for k in range(KC):
                        nc.tensor.matmul(
                            hp[:],
                            tsT[:, k, t * 128 : (t + 1) * 128],
                            w1t_sb(k),
                            start=(k == 0),
                            stop=False,
                        )
                    nc.tensor.matmul(
                        hp[:],
                        cov2[32 * j : 32 * j + 2, t * 128 : (t + 1) * 128],
                        cov2[32 * j : 32 * j + 2, Lp : Lp + H],
                        start=False,
                        stop=True,
                        tile_position=(32 * j, 0),
                    )
                    if ti >= NH and ti % 2 == 0:
                        # make ACT observe the DVE tick that released the next
                        # 2 ht buffers: reduce(ti-2) >= reduce(ti-NH+1)
                        plg, pt = red_hist[ti - 2]
                        col = 2 + (ti % 126)
                        nc.scalar.activation(
                            scr_act[0:1, col : col + 1],
                            plg[0:1, pt : pt + 1],
                            mybir.ActivationFunctionType.Copy,
                        )
                    nc.scalar.activation(
                        ht[:], hp[:], mybir.ActivationFunctionType.Tanh
                    )
                    # in-place: ht *= w2 (broadcast rows), then row-sum
                    nc.vector.tensor_tensor(
                        ht[:], ht[:], w2bc_sb, mybir.AluOpType.mult
                    )
                    nc.vector.tensor_reduce(
                        logits[:, t : t + 1],
                        ht[:],
                        axis=mybir.AxisListType.X,
                        op=mybir.AluOpType.add,
                    )
                    red_hist.append((logits, t))
                    ti += 1

                # ---- softmax tail (masked, bounded-exp)
                expv = consts.tile([128, 32], F32, tag=f"ex{j}")
                nc.scalar.activation(
                    expv[:, 0:nt],
                    logits[:, 0:nt],
                    mybir.ActivationFunctionType.Exp,
                    bias=bexp_sb,
                )
                attw = consts.tile([128, 32], F32, tag=f"aw{j}")
                nc.vector.tensor_tensor(
                    attw[:, 0:nt], expv[:, 0:nt], mask_sb(j, nt),
                    mybir.AluOpType.mult,
                )
                nc.tensor.matmul(
                    psA[0:1, 128 : 128 + nt], ones_col, attw[:, 0:nt],
                    start=True, stop=True,
                )
                den = consts.tile([1, 2], F32, tag=f"dn{j}")
                nc.vector.tensor_reduce(
                    den[0:1, 0:1], psA[0:1, 128 : 128 + nt],
                    axis=mybir.AxisListType.X, op=mybir.AluOpType.add,
                )
                nc.tensor.matmul(
                    psB[:, 0:1], ones_row, den[0:1, 0:1], start=True, stop=True
                )
                rcp = consts.tile([128, 2], F32, tag=f"rc{j}")
                nc.vector.reciprocal(rcp[:, 0:1], psB[:, 0:1])
                nc.vector.tensor_tensor(
                    attw[:, 0:nt],
                    attw[:, 0:nt],
                    rcp[:, 0:1].to_broadcast((128, nt)),
                    mybir.AluOpType.mult,
                )
                attb = consts.tile([128, 32], BF16, tag=f"ab{j}")
                nc.vector.tensor_copy(attb[:, 0:nt], attw[:, 0:nt])

                # attention out: PE-transpose [128, nt] -> [nt, 128], then DMA
                nc.tensor.matmul(
                    psA[0:nt, 0:128], attw[:, 0:nt], ident_sb,
                    is_transpose=True, start=True, stop=True,
                )
                atT = consts.tile([32, 128], F32, tag=f"at{j}")
                nc.vector.tensor_copy(atT[0:nt, :], psA[0:nt, 0:128])
                nc.gpsimd.dma_start(out=att_d[j][:], in_=atT[0:nt, :])

                # context: ctx[c] = sum_l att[l] * text[l, c].  Deferred so
                # the PE can start the next slot's h-matmuls while this
                # slot's softmax tail finishes on ACT/DVE.
                def emit_ctx(j=j, nt=nt, attb=attb, nat=nat):
                    oc = 504 + j
                    nc.tensor.matmul(
                        psA[0:1, oc : oc + 1], nat[0:1, 0, 0:1], nat[0:1, 0, 0:1],
                        start=True, stop=True,
                    )
                    cx = ctxps[j % 2]
                    for t in range(nt):
                        nc.tensor.matmul(
                            cx[:],
                            attb[:, t : t + 1],
                            nat[:, t, :],
                            start=(t == 0),
                            stop=(t == nt - 1),
                        )
                    cxs = consts.tile(
                        [1, FT], BF16, name=f"cs{j}", tag=f"cs{j}"
                    )
                    nc.vector.tensor_copy(cxs[:], cx[:])
                    nc.gpsimd.dma_start(out=ctx_d[j : j + 1, :], in_=cxs[:])

                pending_ctx.append(emit_ctx)

            for fn in pending_ctx:
                fn()

    return nc


def prepare(text_states, summary_current_state, coverage, W1, b1, W2, b2, text_length):
    B, L, ft = text_states.shape
    assert (ft, L) == (FT, 4096) and B == 32

    text_states = np.asarray(text_states, dtype=np.float32)
    summary_current_state = np.asarray(summary_current_state, dtype=np.float32)
    coverage = np.asarray(coverage, dtype=np.float32)
    W1 = np.asarray(W1, dtype=np.float32)
    b1 = np.asarray(b1, dtype=np.float32)
    W2 = np.asarray(W2, dtype=np.float32)
    b2 = np.asarray(b2, dtype=np.float32)
    lens = np.asarray(text_length).astype(np.int64)

    nslots = B // N_CORES

    # length-sorted assignment: slot j holds ranks [8j, 8j+8), one per core
    order = np.argsort(-lens, kind="stable")
    assign = order.reshape(nslots, N_CORES)  # assign[j, i] = batch of core i slot j
    Lps = [
        max(128, int(np.ceil(lens[assign[j]].max() / 128.0) * 128))
        for j in range(nslots)
    ]

    # softmax upper bound: |logit| <= sum|W2| with margin; b2 folded into exp
    Bc = float(np.abs(W2).sum()) + 1.0
    bexp = float(b2.reshape(-1)[0]) - Bc

    nc = build_program(Lps)

    # per-batch bias vectors (tiny): bias_b = W1s @ ss_b + b1
    W1t = W1[:, :FT]
    W1s = W1[:, FT : FT + FT]
    w1c = W1[:, -1]
    biases = summary_current_state @ W1s.T + b1[None, :]  # [B, H] fp32
    w1c_bf = w1c.astype(ml_dtypes.bfloat16)

    constsf = np.zeros((128, CF_COLS), np.float32)
    constsf[:, CF_ONES : CF_ONES + 129] = 1.0
    constsf[:, CF_IDENT : CF_IDENT + 128] = np.eye(128, dtype=np.float32)
    constsf[:, CF_BEXP] = bexp

    constsb = np.zeros((128, CB_COLS), ml_dtypes.bfloat16)
    constsb[:, CB_W1T : CB_W1T + KC * H] = (
        np.ascontiguousarray(W1t.T)
        .reshape(KC, 128, H)
        .transpose(1, 0, 2)
        .reshape(128, KC * H)
        .astype(ml_dtypes.bfloat16)
    )
    constsb[:, CB_W2 : CB_W2 + H] = np.broadcast_to(W2.reshape(1, H), (128, H)).astype(
        ml_dtypes.bfloat16
    )

    iota = np.arange(32)[None, :] * 128 + np.arange(128)[:, None]  # [128, 32]

    in_maps = []
    for i in range(N_CORES):
        cfi = constsf.copy()
        cov2 = np.zeros((128, 4608), ml_dtypes.bfloat16)
        m = {"constsb": constsb, "constsf": cfi, "cov2": cov2}
        for j in range(nslots):
            b = int(assign[j, i])
            Lp = Lps[j]
            cfi[:, CF_MASK + j * 32 : CF_MASK + (j + 1) * 32] = (
                iota < int(lens[b])
            ).astype(np.float32)
            cov2[32 * j, 0:Lp] = coverage[b, 0, :Lp].astype(ml_dtypes.bfloat16)
            cov2[32 * j, Lp : Lp + H] = w1c_bf
            cov2[32 * j + 1, 0:Lp] = 1.0
            cov2[32 * j + 1, Lp : Lp + H] = biases[b].astype(ml_dtypes.bfloat16)
            tb = text_states[b, :Lp, :].astype(ml_dtypes.bfloat16)
            m[f"text{j}"] = tb
            m[f"ttex{j}"] = np.ascontiguousarray(tb.T)
        in_maps.append(m)

    return nc, in_maps, (assign, Lps, B, L)


def postprocess(outs, meta):
    assign, Lps, B, L = meta
    nslots = B // N_CORES
    context = np.zeros((B, FT), np.float32)
    attention = np.zeros((B, 1, L), np.float32)
    for i in range(N_CORES):
        for j in range(nslots):
            b = int(assign[j, i])
            context[b] = outs[i]["ctx"][j].astype(np.float32)
            attention[b, 0, : Lps[j]] = outs[i][f"att{j}"].reshape(-1)
    return context, attention


def kernel(text_states, summary_current_state, coverage, W1, b1, W2, b2, text_length):
    nc, in_maps, meta = prepare(
        text_states, summary_current_state, coverage, W1, b1, W2, b2, text_length
    )
    res = run_bass_kernel_spmd(nc, in_maps, list(range(N_CORES)))
    global LAST_RESULT
    LAST_RESULT = res
    return postprocess(res.results, meta)
